# revision 2
# baseline (speedup 1.0000x reference)
"""LocalExpansion (7x7 unfold) Trainium2 Bass kernel.

Full input x: [2, 8, 2304, 64] f32 (B=2, heads=8, N=48*48, D=64).
Full output:  [2, 8, 2304, 49, 64] f32 — out[b,h,y*W+x,i*7+j,:] =
x_img[b,h,y+i-3,x+j-3,:] with zero fill outside the 48x48 image.

Strategy (pure DMA, memory-regime):
- batch*heads = 16 images, 2 per core across 8 NeuronCores.
- Per core: zero-pad each 48x48x64 image into SBUF as 54 rows
  (one padded row per partition, 54*64 floats = 13824 B). Image 0 on
  partitions 0-53 (even-SDMA-engine half), image 1 on partitions
  64-117 (odd half) so concurrent DMAs load all 16 SDMA engines.
- For each filter row i (7 of them) one 3D DMA writes the whole
  [48 y, 48 x, 7*64 floats] slab: src is an overlapping sliding
  window (x stride 64 floats < 448-float element) read from SBUF,
  dst is strided DRAM with 1792 B contiguous chunks. Boundary zeros
  come for free from the padded SBUF image.
HBM traffic per core = 57.8 MB writes + 1.2 MB reads (~roofline).
"""

import numpy as np

KH, KW = 7, 7
H, W, D = 48, 48, 64
PH, PW = H + 6, W + 6          # 54x54 padded image
ROW = PW * D                   # floats per padded row (one SBUF partition)
N = H * W                      # 2304
K = KH * KW                    # 49
IMG_OUT = N * K * D            # floats per image output
IMGS_PER_CORE = 2
N_CORES = 8
BASES = (0, 64)                # SBUF base partitions per image

_CACHE = {}


def _build_nc():
    import concourse.bass as bass
    import concourse.mybir as mybir

    nc = bass.Bass(trn_type="TRN2")
    x = nc.dram_tensor("x", [IMGS_PER_CORE, N, D], mybir.dt.float32,
                       kind="ExternalInput")
    out = nc.dram_tensor("out", [IMGS_PER_CORE, N, K, D], mybir.dt.float32,
                         kind="ExternalOutput")

    with (
        nc.sbuf_tensor("pad", [128, ROW], mybir.dt.float32) as pad,
        nc.semaphore("ld0") as ld0,
        nc.semaphore("ld1") as ld1,
        nc.semaphore("ms") as ms,
        nc.semaphore("st") as st,
    ):
        # Zero the whole padded buffer once (pad strips stay zero), then
        # load both images into the padded interiors.
        nc.vector.memset(
            bass.AP(pad, 0, [[ROW, 128], [1, ROW]]), 0.0
        ).then_inc(ms, 1)
        # One load per ring. Loads are cross-wired (sync loads img1, scalar
        # loads img0): the DMA engines drain the SP-dispatched load first
        # (SP sequencer dispatch beats ACT), so the intrinsically slower
        # scalar slab stream gets the earlier-finishing load and both
        # queues' finish times balance.
        for im, ring, sem in ((1, nc.sync, ld1), (0, nc.scalar, ld0)):
            bp = BASES[im]
            ring.wait_ge(ms, 1)
            ring.dma_start(
                out=bass.AP(pad, (bp + 3) * ROW + 3 * D, [[ROW, H], [1, W * D]]),
                in_=bass.AP(x, im * N * D, [[W * D, H], [1, W * D]]),
            ).then_inc(sem, 16)

        nc.sync.wait_ge(ld0, 16)
        nc.scalar.wait_ge(ld1, 16)

        # 7 filter-row slabs per image. Ring = image (img0 on sync, img1 on
        # scalar) so the two HWDGE rings carry the two SBUF partition halves
        # concurrently — measured ~1.75x faster than alternating rings per i.
        n_st = 0
        for i in range(KH):
            for im in range(IMGS_PER_CORE):
                ring = nc.sync if im == 0 else nc.scalar
                bp = BASES[im]
                ring.dma_start(
                    out=bass.AP(
                        out,
                        im * IMG_OUT + i * KW * D,
                        [[W * K * D, H], [K * D, W], [1, KW * D]],
                    ),
                    in_=bass.AP(
                        pad,
                        (bp + i) * ROW,
                        [[ROW, H], [D, W], [1, KW * D]],
                    ),
                ).then_inc(st, 16)
                n_st += 16
        nc.sync.wait_ge(st, n_st)
        nc.scalar.wait_ge(st, n_st)
    return nc


def kernel(x, height=48, width=48):
    from concourse.bass_utils import run_bass_kernel_spmd

    x = np.asarray(x)
    b, nh = x.shape[0], x.shape[1]
    xi = np.ascontiguousarray(x.reshape(b * nh, N, D))
    in_maps = [
        {"x": np.ascontiguousarray(xi[IMGS_PER_CORE * c: IMGS_PER_CORE * (c + 1)])}
        for c in range(N_CORES)
    ]
    if "nc" not in _CACHE:
        _CACHE["nc"] = _build_nc()
    res = run_bass_kernel_spmd(_CACHE["nc"], in_maps, core_ids=list(range(N_CORES)))
    y = np.stack([res.results[c]["out"] for c in range(N_CORES)])
    return y.reshape(b, nh, N, K, D).astype(np.float32, copy=False)



# revision 3
# speedup vs baseline: 1.1739x; 1.1739x over previous
"""LocalExpansion (7x7 unfold) Trainium2 Bass kernel.

Full input x: [2, 8, 2304, 64] f32 (B=2, heads=8, N=48*48, D=64).
Full output:  [2, 8, 2304, 49, 64] f32 — out[b,h,y*W+x,i*7+j,:] =
x_img[b,h,y+i-3,x+j-3,:] with zero fill outside the 48x48 image.

Strategy (pure DMA, memory-regime):
- batch*heads = 16 images, 2 per core across 8 NeuronCores.
- Per core: zero-pad each 48x48x64 image into SBUF as 54 rows
  (one padded row per partition, 54*64 floats = 13824 B). Image 0 on
  partitions 0-53 (even-SDMA-engine half), image 1 on partitions
  64-117 (odd half) so concurrent DMAs load all 16 SDMA engines.
- For each filter row i (7 of them) one 3D DMA writes the whole
  [48 y, 48 x, 7*64 floats] slab: src is an overlapping sliding
  window (x stride 64 floats < 448-float element) read from SBUF,
  dst is strided DRAM with 1792 B contiguous chunks. Boundary zeros
  come for free from the padded SBUF image.
HBM traffic per core = 57.8 MB writes + 1.2 MB reads (~roofline).
"""

import numpy as np

KH, KW = 7, 7
H, W, D = 48, 48, 64
PH, PW = H + 6, W + 6          # 54x54 padded image
ROW = PW * D                   # floats per padded row (one SBUF partition)
N = H * W                      # 2304
K = KH * KW                    # 49
IMG_OUT = N * K * D            # floats per image output
IMGS_PER_CORE = 2
N_CORES = 8
BASES = (0, 64)                # SBUF base partitions per image

_CACHE = {}


def _build_nc():
    import concourse.bass as bass
    import concourse.mybir as mybir

    nc = bass.Bass(trn_type="TRN2")
    x = nc.dram_tensor("x", [IMGS_PER_CORE, N, D], mybir.dt.float32,
                       kind="ExternalInput")
    out = nc.dram_tensor("out", [IMGS_PER_CORE, N, K, D], mybir.dt.float32,
                         kind="ExternalOutput")

    with (
        nc.sbuf_tensor("pad", [128, ROW], mybir.dt.float32) as pad,
        nc.semaphore("ld0") as ld0,
        nc.semaphore("ld1") as ld1,
        nc.semaphore("ms") as ms,
        nc.semaphore("st") as st,
    ):
        # Zero the whole padded buffer once (pad strips stay zero), then
        # load both images into the padded interiors.
        nc.vector.memset(
            bass.AP(pad, 0, [[ROW, 128], [1, ROW]]), 0.0
        ).then_inc(ms, 1)
        # One load per ring so both rings start right after the memset, and
        # each ring's slabs gate only on its own image's load (img0 slabs
        # read only img0's pad rows).
        for im, ring, sem in ((0, nc.sync, ld0), (1, nc.scalar, ld1)):
            bp = BASES[im]
            ring.wait_ge(ms, 1)
            ring.dma_start(
                out=bass.AP(pad, (bp + 3) * ROW + 3 * D, [[ROW, H], [1, W * D]]),
                in_=bass.AP(x, im * N * D, [[W * D, H], [1, W * D]]),
            ).then_inc(sem, 16)

        nc.sync.wait_ge(ld0, 16)
        nc.scalar.wait_ge(ld1, 16)

        # 7 filter-row slabs per image. Ring = image (img0 on sync, img1 on
        # scalar) so the two HWDGE rings carry the two SBUF partition halves
        # concurrently — measured ~1.75x faster than alternating rings per i.
        n_st = 0
        for i in range(KH):
            for im in range(IMGS_PER_CORE):
                ring = nc.sync if im == 0 else nc.scalar
                bp = BASES[im]
                ring.dma_start(
                    out=bass.AP(
                        out,
                        im * IMG_OUT + i * KW * D,
                        [[W * K * D, H], [K * D, W], [1, KW * D]],
                    ),
                    in_=bass.AP(
                        pad,
                        (bp + i) * ROW,
                        [[ROW, H], [D, W], [1, KW * D]],
                    ),
                ).then_inc(st, 16)
                n_st += 16
        nc.sync.wait_ge(st, n_st)
        nc.scalar.wait_ge(st, n_st)
    return nc


def kernel(x, height=48, width=48):
    from concourse.bass_utils import run_bass_kernel_spmd

    x = np.asarray(x)
    b, nh = x.shape[0], x.shape[1]
    xi = np.ascontiguousarray(x.reshape(b * nh, N, D))
    in_maps = [
        {"x": np.ascontiguousarray(xi[IMGS_PER_CORE * c: IMGS_PER_CORE * (c + 1)])}
        for c in range(N_CORES)
    ]
    if "nc" not in _CACHE:
        _CACHE["nc"] = _build_nc()
    res = run_bass_kernel_spmd(_CACHE["nc"], in_maps, core_ids=list(range(N_CORES)))
    y = np.stack([res.results[c]["out"] for c in range(N_CORES)])
    return y.reshape(b, nh, N, K, D).astype(np.float32, copy=False)

